# revision 1
# baseline (speedup 1.0000x reference)
"""GPT2 decode-step attention (B=32, q_len=1, S_past=4095, H=2048, NH=16, HD=128)
as a Bass/Tile kernel on 8 trn2 NeuronCores.

Sharding: tensor-parallel over heads — core i owns heads (2i, 2i+1), i.e. the
256-wide column slice [256*i, 256*i+256) of the hidden dim. Each core:
  - computes q/k/v projections for its two heads (full batch),
  - streams its slice of the KV cache (K pre-transposed on host to [b, d, s],
    V repacked to [b, si, so, d] blocks) and does the attention,
  - multiplies by its 256 rows of W_proj, producing a partial [32, 2048].
Host sums the 8 partials and adds b_proj (the "c_proj all-reduce").

Softmax runs without max-subtraction: scores = q.k/sqrt(128) are bounded by a
few units for any plausible inputs at these scales (inputs here give |s|<3.5),
so exp() is safe in fp32 and the result is mathematically identical.

The appended (new) token is handled algebraically: the padded score slot
contributes exp(0)=1 to each row-sum (subtracted at the end) and zero to ctx
(its V row is zero); the true new-token term e_new*v_new and the +e_new
denominator correction are applied once at the end in the [d, pair] domain.
"""

import math
import sys

import numpy as np

for _p in ("/opt/trn_rl_repo",):
    if _p not in sys.path:
        sys.path.append(_p)

import concourse.bass as bass  # noqa: E402
import concourse.tile as tile  # noqa: E402
from concourse import bacc, mybir  # noqa: E402
from concourse.masks import make_identity  # noqa: E402

F32 = mybir.dt.float32
AF = mybir.ActivationFunctionType

# Full-problem dimensions (hardcoded per spec).
B = 32          # batch
H = 2048        # hidden
NH = 16         # heads total
HD = 128        # head dim
DH2 = 2 * HD    # two heads per core
S_PAST = 4095
SO = 32         # s-outer blocks; S = SO*128 = 4096 = S_PAST + 1 (new token slot)
N_CORES = 8
P = 128
SCALE = 1.0 / math.sqrt(HD)


def build_nc(b=B, so=SO, h=H, n_cores=N_CORES):
    """Build the per-core Bass program. All 8 cores run the same program on
    different (pre-sliced) data."""
    s = so * P
    nko = h // P  # contraction chunks for the qkv projection
    nc = bacc.Bacc("TRN2", target_bir_lowering=False, debug=False,
                   num_devices=n_cores)

    kt = nc.dram_tensor("kt", [b, DH2, s], F32, kind="ExternalInput")
    vr = nc.dram_tensor("vr", [b, P, so, DH2], F32, kind="ExternalInput")
    xx = nc.dram_tensor("xx", [b, h], F32, kind="ExternalInput")
    wqkv = nc.dram_tensor("wqkv", [h, 3 * DH2], F32, kind="ExternalInput")
    bqkv = nc.dram_tensor("bqkv", [3 * DH2], F32, kind="ExternalInput")
    wp = nc.dram_tensor("wp", [DH2, h], F32, kind="ExternalInput")
    out = nc.dram_tensor("out", [b, h], F32, kind="ExternalOutput")

    add = mybir.AluOpType.add

    with tile.TileContext(nc) as tc:
        with (
            tc.tile_pool(name="singles", bufs=1) as singles,
            tc.tile_pool(name="wpool", bufs=3) as wpool,
            tc.tile_pool(name="kvpool", bufs=2) as kvpool,
            tc.tile_pool(name="epool", bufs=3) as epool,
            tc.tile_pool(name="rowpool", bufs=3) as rowpool,
            tc.tile_pool(name="psum", bufs=2, space="PSUM") as psum,
            tc.tile_pool(name="psum1", bufs=1, space="PSUM") as psum1,
        ):
            # ---------------- constants / small loads ----------------
            ident = singles.tile([P, P], F32)
            make_identity(nc, ident)
            ones_col = singles.tile([P, 1], F32)
            nc.vector.memset(ones_col, 1.0)
            ones_row = singles.tile([1, P], F32)
            nc.vector.memset(ones_row, 1.0)

            x_sb = singles.tile([b, h], F32)
            nc.sync.dma_start(out=x_sb[:], in_=xx.ap())
            wp_sb = singles.tile([P, 2, h], F32)
            nc.sync.dma_start(out=wp_sb[:],
                              in_=wp.ap().rearrange("(c d) n -> d c n", d=P))
            b6 = singles.tile([4, P], F32)  # q0,q1,k0,k1 bias rows
            nc.sync.dma_start(out=b6[:],
                              in_=bqkv.ap().rearrange("(c p) -> c p", p=P)[0:4, :])
            bv_row = singles.tile([1, DH2], F32)  # v bias as a row
            nc.sync.dma_start(out=bv_row[:],
                              in_=bqkv.ap().rearrange("(a d) -> a d", a=3)[2:3, :])

            ps_b = psum.tile([P, 4], F32, tag="C")
            nc.tensor.transpose(ps_b[:], b6[:], ident[0:4, 0:4])
            bT = singles.tile([P, 4], F32)  # per-partition biases: q0,q1,k0,k1
            nc.vector.tensor_copy(out=bT[:], in_=ps_b[:])

            # ---------------- x^T (PE transpose, 128-col chunks) ----------
            xT = singles.tile([P, nko, b], F32)
            for ko in range(nko):
                ps_x = psum.tile([P, b], F32, tag="C")
                nc.tensor.transpose(ps_x[:], x_sb[:, ko * P:(ko + 1) * P],
                                    ident[0:b, 0:b])
                nc.vector.tensor_copy(out=xT[:, ko, :], in_=ps_x[:])

            # ---------------- qkv projection ----------------
            ps_q0 = psum.tile([P, b], F32, tag="A")
            ps_q1 = psum.tile([P, b], F32, tag="A")
            ps_k0 = psum.tile([P, b], F32, tag="B")
            ps_k1 = psum.tile([P, b], F32, tag="B")
            ps_v = psum.tile([b, DH2], F32, tag="C")
            for ko in range(nko):
                wc = wpool.tile([P, 3 * DH2], F32, tag="wc")
                nc.sync.dma_start(out=wc[:], in_=wqkv.ap()[ko * P:(ko + 1) * P, :])
                st, sp = ko == 0, ko == nko - 1
                rx = xT[:, ko, :]
                nc.tensor.matmul(ps_q0[:], lhsT=wc[:, 0:128], rhs=rx, start=st, stop=sp)
                nc.tensor.matmul(ps_q1[:], lhsT=wc[:, 128:256], rhs=rx, start=st, stop=sp)
                nc.tensor.matmul(ps_k0[:], lhsT=wc[:, 256:384], rhs=rx, start=st, stop=sp)
                nc.tensor.matmul(ps_k1[:], lhsT=wc[:, 384:512], rhs=rx, start=st, stop=sp)
                nc.tensor.matmul(ps_v[:], lhsT=rx, rhs=wc[:, 512:768], start=st, stop=False)
            # + v bias (broadcast over batch rows via K=1 matmul)
            nc.tensor.matmul(ps_v[:], lhsT=ones_row[:, 0:b], rhs=bv_row[:],
                             start=False, stop=True)

            qT = singles.tile([P, 2, b], F32)
            kTn = singles.tile([P, 2, b], F32)
            nc.vector.tensor_scalar_add(out=qT[:, 0, :], in0=ps_q0[:], scalar1=bT[:, 0:1])
            nc.vector.tensor_scalar_add(out=qT[:, 1, :], in0=ps_q1[:], scalar1=bT[:, 1:2])
            nc.vector.tensor_scalar_add(out=kTn[:, 0, :], in0=ps_k0[:], scalar1=bT[:, 2:3])
            nc.vector.tensor_scalar_add(out=kTn[:, 1, :], in0=ps_k1[:], scalar1=bT[:, 3:4])
            vnew = singles.tile([b, DH2], F32)
            nc.vector.tensor_copy(out=vnew[:], in_=ps_v[:])

            # new-token scores for all (h, b): e_new = exp(q.k_new * scale)
            ps_en = psum.tile([1, 2 * b], F32, tag="A")
            for hh in range(2):
                prod = rowpool.tile([P, b], F32, tag="prod")
                nc.vector.tensor_mul(out=prod[:], in0=qT[:, hh, :], in1=kTn[:, hh, :])
                nc.tensor.matmul(ps_en[0:1, hh * b:(hh + 1) * b], lhsT=ones_col[:],
                                 rhs=prod[:], start=True, stop=True)
            en_row = singles.tile([1, 2 * b], F32)
            nc.scalar.activation(out=en_row[:], in_=ps_en[:], func=AF.Exp, scale=SCALE)

            # v_new^T: [d, pair] columns for the end-phase correction
            vnewT = singles.tile([P, 2 * b], F32)
            for hh in range(2):
                ps_vt = psum.tile([P, b], F32, tag="C")
                nc.tensor.transpose(ps_vt[:], vnew[:, hh * HD:(hh + 1) * HD],
                                    ident[0:b, 0:b])
                nc.vector.tensor_copy(out=vnewT[:, hh * b:(hh + 1) * b], in_=ps_vt[:])

            # ---------------- attention main loop ----------------
            ctxT = singles.tile([P, 2 * b], F32)          # [d, pair] unnormalized ctx
            ps_dens = psum1.tile([1, 2 * b], F32, tag="D")     # per-pair raw denominators
            for bb in range(b):
                ktt = []
                for hh in range(2):
                    t = kvpool.tile([P, s], F32, tag=f"kt{hh}")
                    nc.sync.dma_start(out=t[:], in_=kt.ap()[bb, hh * P:(hh + 1) * P, :])
                    ktt.append(t)
                vt = kvpool.tile([P, so, DH2], F32, tag="v")
                nc.scalar.dma_start(out=vt[:], in_=vr.ap()[bb])

                for hh in range(2):
                    pair = hh * b + bb
                    ps_sc = psum.tile([P, so], F32, tag="A")
                    for j in range(so):
                        nc.tensor.matmul(ps_sc[:, j:j + 1],
                                         lhsT=ktt[hh][:, j * P:(j + 1) * P],
                                         rhs=qT[:, hh, bb:bb + 1],
                                         start=True, stop=True)
                    e_sb = epool.tile([P, so], F32, tag="e")
                    rs = rowpool.tile([P, 1], F32, tag="rs")
                    nc.scalar.activation(out=e_sb[:], in_=ps_sc[:], func=AF.Exp,
                                         scale=SCALE, accum_out=rs[:])
                    # raw denominator (includes +1 from the zero pad slot)
                    nc.tensor.matmul(ps_dens[0:1, pair:pair + 1], lhsT=rs[:],
                                     rhs=ones_col[:], start=True, stop=True)
                    # ctx = E^T V accumulated over the 32 blocks
                    ps_cd = psum.tile([1, HD], F32, tag="B")
                    for j in range(so):
                        nc.tensor.matmul(ps_cd[:], lhsT=e_sb[:, j:j + 1],
                                         rhs=vt[:, j, hh * HD:(hh + 1) * HD],
                                         start=(j == 0), stop=(j == so - 1))
                    cdr = rowpool.tile([1, HD], F32, tag="cdr")
                    nc.vector.tensor_copy(out=cdr[:], in_=ps_cd[:])
                    ps_ct = psum.tile([P, 1], F32, tag="C")
                    nc.tensor.transpose(ps_ct[:], cdr[:], ident[0:1, 0:1])
                    nc.vector.tensor_copy(out=ctxT[:, pair:pair + 1], in_=ps_ct[:])

            # ---------------- end phase: new token, normalize, project -----
            dens = singles.tile([1, 2 * b], F32)
            nc.vector.tensor_copy(out=dens[:], in_=ps_dens[:])
            nc.vector.tensor_add(out=dens[:], in0=dens[:], in1=en_row[:])
            nc.vector.tensor_scalar_add(out=dens[:], in0=dens[:], scalar1=-1.0)
            recip = singles.tile([1, 2 * b], F32)
            nc.vector.reciprocal(out=recip[:], in_=dens[:])

            # broadcast e_new over partitions; ctxT += vnewT * e_new
            ps_enb = psum.tile([P, 2 * b], F32, tag="A")
            nc.tensor.matmul(ps_enb[:], lhsT=ones_row[:], rhs=en_row[:],
                             start=True, stop=True)
            nc.vector.tensor_mul(out=vnewT[:], in0=vnewT[:], in1=ps_enb[:])
            nc.vector.tensor_add(out=ctxT[:], in0=ctxT[:], in1=vnewT[:])
            # broadcast 1/denom; ctxT *= recip
            ps_rb = psum.tile([P, 2 * b], F32, tag="B")
            nc.tensor.matmul(ps_rb[:], lhsT=ones_row[:], rhs=recip[:],
                             start=True, stop=True)
            nc.vector.tensor_mul(out=ctxT[:], in0=ctxT[:], in1=ps_rb[:])

            # output projection: out[b, n] = sum_h ctxT[:, h-cols].T @ wp[h]
            out_sb = singles.tile([b, h], F32)
            nt = h // 512
            for n in range(nt):
                ps_o = psum.tile([b, 512], F32, tag=("A" if n % 2 == 0 else "B"))
                for hh in range(2):
                    nc.tensor.matmul(ps_o[:], lhsT=ctxT[:, hh * b:(hh + 1) * b],
                                     rhs=wp_sb[:, hh, n * 512:(n + 1) * 512],
                                     start=(hh == 0), stop=(hh == 1))
                nc.vector.tensor_copy(out=out_sb[:, n * 512:(n + 1) * 512], in_=ps_o[:])
            nc.sync.dma_start(out=out.ap(), in_=out_sb[:])

    nc.finalize()
    return nc


_NC_CACHE = {}


def _get_nc():
    key = (B, SO, H, N_CORES)
    if key not in _NC_CACHE:
        _NC_CACHE[key] = build_nc()
    return _NC_CACHE[key]


def make_in_maps(x, past_key, past_value, W_attn, b_attn, W_proj):
    """Host-side shard + repack: per-core input dict."""
    x = np.ascontiguousarray(np.asarray(x, np.float32).reshape(B, H))
    past_key = np.asarray(past_key, np.float32)
    past_value = np.asarray(past_value, np.float32)
    W_attn = np.asarray(W_attn, np.float32)
    b_attn = np.asarray(b_attn, np.float32)
    W_proj = np.asarray(W_proj, np.float32)

    s = SO * P
    in_maps = []
    for i in range(N_CORES):
        c0 = DH2 * i
        kt = np.zeros((B, DH2, s), np.float32)
        kt[:, :, :S_PAST] = past_key[:, :, c0:c0 + DH2].transpose(0, 2, 1)
        vtmp = np.zeros((B, s, DH2), np.float32)
        vtmp[:, :S_PAST] = past_value[:, :, c0:c0 + DH2]
        vr = np.ascontiguousarray(
            vtmp.reshape(B, SO, P, DH2).transpose(0, 2, 1, 3))
        wqkv = np.ascontiguousarray(np.concatenate(
            [W_attn[:, c0:c0 + DH2],
             W_attn[:, H + c0:H + c0 + DH2],
             W_attn[:, 2 * H + c0:2 * H + c0 + DH2]], axis=1))
        bq = np.ascontiguousarray(np.concatenate(
            [b_attn[c0:c0 + DH2],
             b_attn[H + c0:H + c0 + DH2],
             b_attn[2 * H + c0:2 * H + c0 + DH2]]))
        wpc = np.ascontiguousarray(W_proj[c0:c0 + DH2, :])
        in_maps.append({"kt": kt, "vr": vr, "xx": x, "wqkv": wqkv,
                        "bqkv": bq, "wp": wpc})
    return in_maps


def kernel(x, past_key, past_value, W_attn, b_attn, W_proj, b_proj):
    from concourse.bass_utils import run_bass_kernel_spmd

    in_maps = make_in_maps(x, past_key, past_value, W_attn, b_attn, W_proj)
    nc = _get_nc()
    res = run_bass_kernel_spmd(nc, in_maps, core_ids=list(range(N_CORES)))
    acc = np.zeros((B, H), np.float32)
    for r in res.results:
        acc += r["out"]
    acc += np.asarray(b_proj, np.float32)[None, :]
    return acc.reshape(B, 1, H)

